# revision 15
# baseline (speedup 1.0000x reference)
"""EdgeGuidance Trainium2 kernel (v3).

Pipeline per image [3,544,960] -> [1,136,240]:
  gray = w.RGB  ->  smooth = gauss5x5(reflect)  ->  gx,gy = sobel(zero-pad)
  mag = sqrt(gx^2+gy^2+1e-6)  ->  4x4 avgpool  ->  sigmoid(5(x-0.2))^2

Linear steps folded into two banded-matrix passes on the PE:
  gx = A_x @ gray @ Bx^T,   gy = A_y @ gray @ By^T
Phase A uses gray as stationary so its output lands transposed ([w, s]);
phase B contracts over w with the B^T band as stationary.

v3 design notes:
  - fp16 PE path (same 10-bit mantissa as tf32/f32r, 2x matmul rate,
    fp16 constants DMA straight to matmul - no rounding copies)
  - input prefetched as 30 contiguous channel-block DMAs over all 3 DMA
    queues (sync/scalar/gpsimd) to engage many SDMA engines at once
  - no-accumulation banding: 5 overlapping K-blocks (<=128 gray rows)
    each produce one 122-row s-block in one matmul
  - gx|gy packed contiguously in PSUM: ONE ACT Square covers both fields
    (engines may read only one PSUM operand per instruction)
  - elementwise spread across DVE / ACT / Pool within their legal op sets

Data parallel over batch: 8 cores x 2 images.
"""

import numpy as np

import concourse.bass as bass
import concourse.tile as tile
from concourse import mybir
from concourse.bass_utils import run_bass_kernel_spmd

F32 = mybir.dt.float32
F32R = mybir.dt.float32r
F16 = mybir.dt.float16
AF = mybir.ActivationFunctionType
ALU = mybir.AluOpType

B_FULL, C, H, W = 16, 3, 544, 960
N_CORES = 8
B_LOC = B_FULL // N_CORES
HP, WP = H // 4, W // 4  # 136, 240

BLUR_K, SIGMA = 5, 1.5
W_R, W_G, W_B = 0.2989, 0.587, 0.114

# ---- vertical blocking: 5 s-blocks of 122 (last 56, padded to 122)
SR = 122
SB = [(SR * i, min(H, SR * (i + 1))) for i in range(5)]
KB = [(max(0, s0 - 3), min(H, s1 + 3)) for s0, s1 in SB]  # gray rows, K<=128
N_WC = 8  # w-chunks of 120 outputs each


def _wj(j):
    return max(0, 120 * j - 4), min(W, 120 * j + 124)


# ---------------------------------------------------------------- numpy bands
def _blur1d():
    x = np.arange(BLUR_K, dtype=np.float64) - (BLUR_K - 1) / 2.0
    g = np.exp(-(x**2) / (2.0 * SIGMA**2))
    return g / g.sum()


def _band_reflect(n, taps):
    r = len(taps) // 2
    m = np.zeros((n, n), dtype=np.float64)
    for s in range(n):
        for d in range(-r, r + 1):
            i = s + d
            if i < 0:
                i = -i
            elif i >= n:
                i = 2 * n - 2 - i
            m[s, i] += taps[d + r]
    return m


def _band_zero(n, taps):
    r = len(taps) // 2
    m = np.zeros((n, n), dtype=np.float64)
    for s in range(n):
        for d in range(-r, r + 1):
            i = s + d
            if 0 <= i < n:
                m[s, i] += taps[d + r]
    return m


def _round_f32r(a):
    b = np.asarray(a, dtype=np.float32).copy()
    v = b.view(np.uint32)
    v += 0x1000
    v &= np.uint32(0xFFFFE000)
    return b


def build_constants():
    g1 = _blur1d()
    vb = _band_reflect(H, g1)
    hb = _band_reflect(W, g1)
    ax = _band_zero(H, [1.0, 2.0, 1.0]) @ vb * W_R
    ay = _band_zero(H, [-1.0, 0.0, 1.0]) @ vb * W_R
    bx = _band_zero(W, [-1.0, 0.0, 1.0]) @ hb
    by = _band_zero(W, [1.0, 2.0, 1.0]) @ hb

    # bandA [128, 5*244]: block i cols [244i,244i+122)=ax.T, +122=ay.T
    band_a = np.zeros((128, 5 * 244), dtype=np.float32)
    for i in range(5):
        s0, s1 = SB[i]
        r0, r1 = KB[i]
        k, ns = r1 - r0, s1 - s0
        band_a[0:k, 244 * i : 244 * i + ns] = ax[s0:s1, r0:r1].T
        band_a[0:k, 244 * i + 122 : 244 * i + 122 + ns] = ay[s0:s1, r0:r1].T

    # bandB [128, 6*120]: (kind, field): kind 0 = j0, 1 = interior, 2 = j7
    band_b = np.zeros((128, 6 * 120), dtype=np.float32)
    for t, m in enumerate((bx, by)):
        for kind, j in ((0, 0), (1, 1), (2, 7)):
            w0, w1 = _wj(j)
            blk = m[120 * j : 120 * j + 120, w0:w1].T  # [mj, 120]
            band_b[0 : w1 - w0, (2 * kind + t) * 120 : (2 * kind + t + 1) * 120] = blk

    p4 = np.zeros((128, 30), dtype=np.float32)
    for wp in range(120):
        p4[wp, wp // 4] = 1.0 / 16.0
    return (_round_f32r(band_a), band_b.astype(np.float16),
            p4.astype(np.float16))


# ------------------------------------------------------------------ bass build
def split_multi_waits(nc):
    """walrus in this container only accepts 1 sync-wait per instruction;
    hoist extra waits onto preceding same-engine NoOps."""
    for fn in nc.m.functions:
        for bb in fn.blocks:
            new_list, changed = [], False
            for ins in bb.instructions:
                si = ins.sync_info
                waits = list(si.on_wait) if si is not None else []
                if len(waits) > 1:
                    changed = True
                    for i, wt in enumerate(waits[:-1]):
                        new_list.append(
                            mybir.InstNoOp(
                                name=f"{ins.name}_ws{i}",
                                engine=ins.engine,
                                bass_nofuse=True,
                                sync_info=mybir.SyncInfo(on_wait=[wt], on_update=[]),
                            )
                        )
                    si.on_wait = [waits[-1]]
                    ins.sync_info = si
                new_list.append(ins)
            if changed:
                bb.instructions = new_list


def build_module():
    nc = bass.Bass("TRN2", target_bir_lowering=False, debug=False)
    x = nc.dram_tensor("x", [B_LOC, C, H, W], F32, kind="ExternalInput").ap()
    ba = nc.dram_tensor("bA", [128, 5 * 244], F32, kind="ExternalInput").ap()
    bb_ = nc.dram_tensor("bB", [128, 6 * 120], F16, kind="ExternalInput").ap()
    p4 = nc.dram_tensor("p4", [128, 30], F16, kind="ExternalInput").ap()
    y = nc.dram_tensor("y", [B_LOC, 1, HP, WP], F32, kind="ExternalOutput").ap()

    with tile.TileContext(nc) as tc:
        with (
            tc.tile_pool(name="const", bufs=1) as cpool,
            tc.tile_pool(name="rgb", bufs=8) as rgbp,
            tc.tile_pool(name="t1", bufs=3) as t1p,
            tc.tile_pool(name="gray", bufs=10) as grayp,
            tc.tile_pool(name="xy", bufs=3) as xyp,
            tc.tile_pool(name="sq", bufs=3) as sqp,
            tc.tile_pool(name="mg", bufs=3) as mgp,
            tc.tile_pool(name="sp", bufs=3) as spp,
            tc.tile_pool(name="outp", bufs=2) as outp,
            tc.tile_pool(name="psA", bufs=1, space="PSUM") as psA,
            tc.tile_pool(name="psB", bufs=1, space="PSUM") as psB,
            tc.tile_pool(name="psP", bufs=2, space="PSUM") as psP,
        ):
            # ---- constants (bandB/p4 fp16 straight from DMA; bandA f32r
            # needs an on-chip rounding copy for the verifier)
            ba_raw = cpool.tile([128, 5 * 244], F32, tag="ba_raw")
            nc.sync.dma_start(ba_raw[:], ba[:])
            ba_t = cpool.tile([128, 5 * 244], F32, tag="ba")
            nc.gpsimd.tensor_copy(ba_t[:].bitcast(F32R), ba_raw[:])
            bb_t = cpool.tile([128, 6 * 120], F16, tag="bb")
            nc.scalar.dma_start(bb_t[:], bb_[:])
            p4_t = cpool.tile([128, 30], F16, tag="p4")
            nc.scalar.dma_start(p4_t[:], p4[:])
            bias_m1 = cpool.tile([128, 1], F32, tag="bm1")
            nc.gpsimd.memset(bias_m1[:], -1.0)
            bias_eps = cpool.tile([128, 1], F32, tag="beps")
            nc.gpsimd.memset(bias_eps[:], 1e-6)

            # ---- prefetch ALL rgb blocks, split between the SWDGE queue
            # (cheap descriptor gen) and the sync HWDGE ring (otherwise
            # idle) so both generators emit concurrently; scalar's ring is
            # kept free for ACT compute
            rgb_t = {}
            for b in range(B_LOC):
                for i in range(5):
                    r0, r1 = KB[i]
                    k = r1 - r0
                    t = rgbp.tile([128, 3 * W], F32, tag="rgb")
                    eng = nc.sync if i % 2 == 1 else nc.gpsimd
                    eng.dma_start(
                        t[0:k, :].rearrange("p (c w) -> p c w", c=3),
                        x[b, :, r0:r1, :].rearrange("c r w -> r c w"),
                    )
                    rgb_t[(b, i)] = t

            for b in range(B_LOC):
                # ---- gray' = (c1 G + R) + c2 B per K-block (f32 DVE stt,
                # f32r-rounded output for phase A)
                gray_t = []
                for i in range(5):
                    r0, r1 = KB[i]
                    k = r1 - r0
                    rgb = rgb_t[(b, i)]
                    tr = rgb[:, 0:W]
                    tg = rgb[:, W : 2 * W]
                    tb = rgb[:, 2 * W : 3 * W]
                    t1 = t1p.tile([128, W], F32, tag="t1")
                    nc.vector.scalar_tensor_tensor(
                        t1[0:k, :], tg[0:k, :], W_G / W_R, tr[0:k, :],
                        op0=ALU.mult, op1=ALU.add,
                    )
                    gt = grayp.tile([128, W], F32, tag="gray")
                    nc.vector.scalar_tensor_tensor(
                        gt[0:k, :].bitcast(F32R), tb[0:k, :], W_B / W_R, t1[0:k, :],
                        op0=ALU.mult, op1=ALU.add,
                    )
                    gray_t.append(gt)

                pooled = {}

                def stage_a(j):
                    w0, w1 = _wj(j)
                    mj = w1 - w0
                    ps = psA.tile([128, 1280], F32, tag="psA")
                    for i in range(5):
                        r0, r1 = KB[i]
                        k = r1 - r0
                        nc.tensor.matmul(
                            ps[0:mj, 256 * i : 256 * i + 244],
                            gray_t[i][0:k, w0:w1].bitcast(F32R),
                            ba_t[0:k, 244 * i : 244 * (i + 1)].bitcast(F32R),
                            start=True, stop=True,
                        )
                    return ps

                def stage_copy(j, ps):
                    w0, w1 = _wj(j)
                    mj = w1 - w0
                    psv = ps[0:mj, :].rearrange("p (b c) -> p b c", b=5)
                    xyx = xyp.tile([128, 610], F16, tag="xyx")
                    xyy = xyp.tile([128, 610], F16, tag="xyy")
                    with nc.allow_low_precision(reason="fp16 PE path"):
                        nc.vector.tensor_copy(
                            xyx[0:mj, :].rearrange("p (b c) -> p b c", b=5),
                            psv[:, :, 0:122],
                        )
                        nc.scalar.copy(
                            xyy[0:mj, :].rearrange("p (b c) -> p b c", b=5),
                            psv[:, :, 122:244],
                        )
                    return xyx, xyy

                def stage_b(j, xyx, xyy):
                    w0, w1 = _wj(j)
                    mj = w1 - w0
                    kind = 0 if j == 0 else (2 if j == N_WC - 1 else 1)
                    # gg: gx [0:512)+[1024:1056), gy [512:1024)+[1056:1088)
                    gg = psB.tile([128, 1088], F32, tag="gg")
                    for f, xyf in enumerate((xyx, xyy)):
                        bT = bb_t[0:mj, (2 * kind + f) * 120 : (2 * kind + f + 1) * 120]
                        nc.tensor.matmul(
                            gg[0:120, 512 * f : 512 * f + 512],
                            bT, xyf[0:mj, 0:512],
                            start=True, stop=True,
                        )
                        nc.tensor.matmul(
                            gg[0:120, 1024 + 32 * f : 1056 + 32 * f],
                            bT, xyf[0:mj, 512:544],
                            start=True, stop=True,
                        )
                    return gg

                def stage_mag(j, gg):
                    # ONE ACT Square covers gx and gy (contiguous psum cols)
                    sq = sqp.tile([128, 1088], F32, tag="sq")
                    nc.scalar.activation(sq[0:120, :], gg[0:120, :], AF.Square)
                    # m2 = sqx + sqy on Pool (SBUF only); eps folded into sqrt
                    m2 = mgp.tile([128, 544], F32, tag="m2")
                    nc.gpsimd.tensor_add(
                        m2[0:120, 0:512], sq[0:120, 0:512], sq[0:120, 512:1024])
                    nc.gpsimd.tensor_add(
                        m2[0:120, 512:544], sq[0:120, 1024:1056],
                        sq[0:120, 1056:1088])
                    mg = mgp.tile([128, 544], F16, tag="mg")
                    with nc.allow_low_precision(reason="fp16 mag"):
                        nc.scalar.activation(
                            mg[0:120, :], m2[0:120, :], AF.Sqrt,
                            bias=bias_eps[0:120, :],
                        )
                    # s-pool: sum groups of 4 over s (fp16, 2x DVE rate)
                    sp = spp.tile([128, 136], F16, tag="sp")
                    with nc.allow_low_precision(reason="fp16 pool"):
                        nc.vector.tensor_reduce(
                            sp[0:120, :],
                            mg[0:120, :].rearrange("p (g f) -> p g f", f=4),
                            axis=mybir.AxisListType.X,
                            op=ALU.add,
                        )
                    return sp

                def stage_pool(j, sp):
                    g = j // 4
                    q = j % 4
                    if q == 0:
                        pooled[g] = psP.tile(
                            [128, 240], F32, tag="pooled", name="pooled"
                        )
                    nc.tensor.matmul(
                        pooled[g][0:128, 30 * q : 30 * q + 30],
                        sp[0:120, 0:128], p4_t[0:120, :],
                        start=True, stop=True,
                    )
                    nc.tensor.matmul(
                        pooled[g][0:8, 120 + 30 * q : 150 + 30 * q],
                        sp[0:120, 128:136], p4_t[0:120, :],
                        start=True, stop=True,
                    )

                def stage_fin(g, ot):
                    pg = pooled.pop(g)
                    nc.scalar.activation(
                        ot[0:128, 120 * g : 120 * g + 120], pg[0:128, 0:120],
                        AF.Sigmoid, bias=bias_m1[0:128, :], scale=5.0,
                    )
                    nc.scalar.activation(
                        ot[0:8, 240 + 120 * g : 360 + 120 * g], pg[0:8, 120:240],
                        AF.Sigmoid, bias=bias_m1[0:8, :], scale=5.0,
                    )

                # software pipeline: A(j) | copy/B/mag(j-1) | pool(j-2)
                ot = outp.tile([128, 480], F32, tag="ot")
                st = {}
                for j in range(N_WC + 2):
                    if j < N_WC:
                        st[j] = [stage_a(j)]
                    if 1 <= j <= N_WC:
                        ps = st[j - 1].pop()
                        xyx, xyy = stage_copy(j - 1, ps)
                        gg = stage_b(j - 1, xyx, xyy)
                        st[j - 1] = [stage_mag(j - 1, gg)]
                    if j >= 2:
                        stage_pool(j - 2, st.pop(j - 2)[0])
                stage_fin(0, ot)
                stage_fin(1, ot)

                # square sigmoid and store
                o2 = outp.tile([128, 480], F32, tag="o2")
                nc.gpsimd.tensor_mul(o2[0:128, 0:240], ot[0:128, 0:240], ot[0:128, 0:240])
                nc.gpsimd.tensor_mul(o2[0:8, 240:480], ot[0:8, 240:480], ot[0:8, 240:480])
                nc.sync.dma_start(y[b, 0, 0:128, :], o2[0:128, 0:240])
                nc.scalar.dma_start(y[b, 0, 128:136, :], o2[0:8, 240:480])

    split_multi_waits(nc)
    return nc


_NC = None
_CONSTS = None
TRACE = False
LAST_EXEC_NS = None


def kernel(**inputs):
    global _NC, _CONSTS, LAST_EXEC_NS
    left_rgb = np.ascontiguousarray(np.asarray(inputs["left_rgb"], dtype=np.float32))
    assert left_rgb.shape == (B_FULL, C, H, W)
    if _NC is None:
        _NC = build_module()
        _CONSTS = build_constants()
    band_a, band_b, p4 = _CONSTS
    in_maps = [
        {
            "x": np.ascontiguousarray(left_rgb[i * B_LOC : (i + 1) * B_LOC]),
            "bA": band_a,
            "bB": band_b,
            "p4": p4,
        }
        for i in range(N_CORES)
    ]
    res = run_bass_kernel_spmd(
        _NC, in_maps, core_ids=list(range(N_CORES)), trace=TRACE
    )
    LAST_EXEC_NS = res.exec_time_ns
    out = np.empty((B_FULL, 1, HP, WP), dtype=np.float32)
    for i in range(N_CORES):
        out[i * B_LOC : (i + 1) * B_LOC] = res.results[i]["y"]
    return out
